# revision 78
# baseline (speedup 1.0000x reference)
"""Trainium2 Bass kernel for nn_BHS_TEST_16724602651186 (gnn_message_passing).

Self-contained: takes FULL inputs (as from reference.setup_inputs()), shards
across 8 NeuronCores internally, returns the FULL (4,4,3) float32 output.

Math (verified against the reference semantics):
  Edge indices are < N, so NNConv message passing only affects s=0 rows.
  With nn1_b1 == 0 and edge_attr >= 0 (asserted at runtime), the edge MLP is
  exactly rank-1:  eh[e] = a_e * relu(W1),  so
      agg[n] = (sum_{e->n} a_e * x0[src_e]) @ Wc,
      Wc[f,h] = sum_k relu(W1)_k * W2[f*H+h, k]    (host-folded).
  All biases (conv_b, gru_b*, nn1_b*) are zero (asserted), h0 == 0 (asserted).
  conv_out[s] = relu(([s==0] u @ Wc) + x[s] @ root_W)
  then a 1-layer GRU over s (batch = nodes), then dueling heads.

Design (v3):
  - dst-node sharding, 1024 nodes/core; packed on-chip layout:
    partitions 0-63 = features of nodes 0-511, partitions 64-127 = features
    of nodes 512-1023 (columns = node within half).
  - Segment-sum: host pre-gathers AND pre-scales x0[src]*a into a padded
    slot grid laid out (tf-partition, chunk, node, level) so a DVE
    tensor_reduce over the innermost (level) axis yields the aggregate
    directly in (t*16+f, node) orientation = exactly the u-term moving
    operand (no transpose, no identity, no separate scale pass).
  - GRU state is stored as h16 = 16*h (Whh/16 folded on host) so the fp8
    copy of the state for the head needs no separate scaling op.
    Gates use 128-row block-diagonal stationaries (one full-width matmul
    per gate per source). n-gate: hh-matmul into PSUM, DVE multiplies by
    r in place, then the ih-matmul accumulates on top (start=False).
  - Dueling head in one fp8 DoubleRow stream: per k-pair g the moving is
    [val1_W8 (64) | adv_W8 (12) | adv_R8 (12)] e4m3 columns and the
    stationary is [y8 (4 s-cols) | ry8 (4)], ry8 = h16 - y8 (error
    feedback). 256 matmuls, out (8,88) per strip, 2 PSUM strips.
    Host recombines rows/cols with the quantization scales; adv gets full
    residual correction (adv_W e4m3 alone would cost 2.3e-2 rel err),
    val1's e4m3 error averages out through the val2/val3 layers (3.8e-3).
  - DMA: sync ring: prm, xT, vg halves (fold/GRU feed); scalar ring: the
    5.8 MB fp8 head weights in 4 chunks.
"""
import numpy as np
import ml_dtypes

import concourse.bacc as bacc
import concourse.mybir as mybir
import concourse.tile as tile
from concourse.bass_utils import run_bass_kernel_spmd

F32 = mybir.dt.float32
BF16 = mybir.dt.bfloat16
F8 = mybir.dt.float8e4
AF = mybir.ActivationFunctionType
ALU = mybir.AluOpType
PM = mybir.MatmulPerfMode
AX = mybir.AxisListType

N, FIN, H, S, E, M = 8192, 16, 64, 4, 131072, 8
NL = N // M            # 1024 dst nodes per core
HF = NL // 2           # 512 columns in packed layout
NJ = 88                # head cols: 64 val1_W8 | 12 adv_W8 | 12 adv_R8
NJP = 96               # moving pair stride padded to a 16-multiple (ISA rule)
KG = HF // 2           # 256 DoubleRow k-pair groups per core
SW, SR, SY = 256.0, 4096.0, 16.0   # W8, adv residual, ys scales

NP_F8 = ml_dtypes.float8_e4m3
LAST_RESULTS = None    # BassKernelResults of the most recent run (for test.py)
_PROGRAM_CACHE = {}
_DEBUG_YS = False


def _bf16(x):
    return np.ascontiguousarray(np.asarray(x, dtype=np.float32)).astype(
        ml_dtypes.bfloat16)


def _f8(x):
    return np.ascontiguousarray(np.asarray(x, dtype=np.float32)).astype(NP_F8)


# ---------------------------------------------------------------- host plan --
def build_plan(edge, edge_attr):
    """Slot layout: for each core, slot level j, node n: the j-th in-edge of
    node n (src index + edge_attr), zero-filled.  Nodes are permuted by
    descending in-degree so the slot grid can be 2-tiered: levels [0, dA)
    cover all nodes, levels [dA, Dp) only the 256 highest-degree nodes
    (blocks 0-1 = partitions 0-31 in tf packing).  The permutation is
    applied consistently to xT and the head weights; the output is a sum
    over nodes so nothing needs un-permuting."""
    src = np.asarray(edge[0], dtype=np.int64)
    dst = np.asarray(edge[1], dtype=np.int64)
    a = np.asarray(edge_attr[:, 0], dtype=np.float32)

    percore = []
    perms = np.zeros((M, NL), dtype=np.int64)
    dA_req, D_req = 1, 1
    for c in range(M):
        lo = c * NL
        mask = (dst >= lo) & (dst < lo + NL)
        src_c, a_c, dstl = src[mask], a[mask], dst[mask] - lo
        deg = np.bincount(dstl, minlength=NL)
        perm = np.argsort(-deg, kind="stable")
        inv = np.empty(NL, dtype=np.int64)
        inv[perm] = np.arange(NL)
        perms[c] = perm
        degs = deg[perm]
        dA_req = max(dA_req, int(degs[256:].max()))
        D_req = max(D_req, int(degs.max()))
        percore.append((src_c, a_c, inv[dstl]))

    dA = (dA_req + 1) // 2 * 2
    Dp = max((D_req + 1) // 2 * 2, dA + 2)

    idxs = np.zeros((M, Dp, NL), dtype=np.int16)
    avals = np.zeros((M, Dp, NL), dtype=np.float32)
    for c in range(M):
        src_c, a_c, dstl = percore[c]
        order = np.argsort(dstl, kind="stable")
        ds = dstl[order]
        starts = np.searchsorted(ds, np.arange(NL))
        occ = np.arange(len(ds)) - starts[ds]
        idxs[c, occ, ds] = src_c[order].astype(np.int16)
        avals[c, occ, ds] = a_c[order]
    return dA, Dp, idxs, avals, perms


# ------------------------------------------------------------- bass program --
def build_program(dA, Dp):
    QA = dA // 2           # levels per tier-A vg chunk (2 chunks)
    DB = Dp - dA           # tier-B levels (blocks 0-1 / partitions 0-31 only)
    nc = bacc.Bacc("TRN2", target_bir_lowering=False, debug=False,
                   num_devices=M, num_swdge_queues=1)
    d = {}
    def din(name, shape, dt=BF16):
        d[name] = nc.dram_tensor(name, list(shape), dt, kind="ExternalInput").ap()
    din("vg", (128, 2 * 128 * QA))
    din("vgb", (32, 128 * DB))
    din("xT", (FIN, S * NL))
    din("prm", (128, 1344))   # [ih-bd(384) | hh-bd/16(384) | wcst(512) | rootw rows0:16 (64)]
    din("wh", (128, KG * 2 * NJP), F8)
    out_d = nc.dram_tensor("partial", [8, NJ], F32, kind="ExternalOutput").ap()
    dbg_d = nc.dram_tensor("dbg_ys", [128, S * HF], mybir.dt.bfloat16,
                           kind="ExternalOutput").ap() if _DEBUG_YS else None
    dbg_x = nc.dram_tensor("dbg_xts", [128, S * HF], mybir.dt.bfloat16,
                           kind="ExternalOutput").ap() if _DEBUG_YS else None
    dbg_u = nc.dram_tensor("dbg_ut", [128, 128], mybir.dt.bfloat16,
                           kind="ExternalOutput").ap() if _DEBUG_YS else None

    mm = nc.tensor.matmul

    with tile.TileContext(nc) as tc:
        with (
            tc.tile_pool(name="sb", bufs=1) as sb,
            tc.tile_pool(name="ps", bufs=1, space="PSUM") as ps,
        ):
            # ---- scalar ring (the fast one, ~390GB/s vs sync's ~130): vg
            # tiers first (ys critical path), then the fp8 head weights
            V = sb.tile([128, 2, QA, 128], BF16, tag="V")
            Vf = V[:].rearrange("p c q n -> p (c q n)")
            CW = 128 * QA
            for c in range(2):
                nc.scalar.dma_start(Vf[:, c * CW:(c + 1) * CW],
                                    d["vg"][:, c * CW:(c + 1) * CW])
            VB = sb.tile([32, DB, 128], BF16, tag="VB")
            nc.scalar.dma_start(VB[:].rearrange("p q n -> p (q n)"), d["vgb"])
            # single DMA: more chunks stall the scalar engine on HWDGE ring
            # credits, delaying the conv relus behind the issue instructions
            wsb = sb.tile([128, KG, 2, NJP], F8, tag="wsb")
            wf = wsb[:].rearrange("p g i j -> p (g i j)")
            nc.scalar.dma_start(wf[:], d["wh"][:])

            # ---- sync ring: prm + xT (small, needed by conv/GRU)
            prm = sb.tile([128, 1344], BF16, tag="prm")
            nc.sync.dma_start(prm[:], d["prm"])
            xTt = sb.tile([FIN, S * NL], BF16, tag="xT")
            nc.sync.dma_start(xTt[:], d["xT"])
            rootw = prm[0:16, 1280:1344]

            # dummy 1-col Tanh pulls the ACT_TABLE_LOAD for the tanh table
            # into the idle DMA-wait window instead of mid-GRU
            warm = sb.tile([128, 4], BF16, tag="warm")
            nc.vector.memset(warm[:], 0.0)
            nc.scalar.activation(warm[:, 0:1], warm[:, 1:2], AF.Tanh)
            nc.scalar.activation(warm[:, 2:3], warm[:, 3:4], AF.Sigmoid)

            # ---- fold: bf16 trees over the level (outer) dim so every add
            # runs on contiguous node columns in DVE 2x mode.  Tier B (the
            # 256 highest-degree nodes, partitions 0-31) folds first, then
            # tier A's two chunks, then the cross adds.
            cur = DB
            while cur > 1:
                half = (cur + 1) // 2
                nch = cur - half
                nc.vector.tensor_tensor(VB[:, 0:nch, :], VB[:, 0:nch, :],
                                        VB[:, half:half + nch, :], ALU.add)
                cur = half
            for c in range(2):
                cur = QA
                while cur > 1:
                    half = (cur + 1) // 2
                    nch = cur - half
                    nc.vector.tensor_tensor(V[:, c, 0:nch, :], V[:, c, 0:nch, :],
                                            V[:, c, half:half + nch, :], ALU.add)
                    cur = half
            ut = sb.tile([128, 128], BF16, tag="ut")
            nc.vector.tensor_tensor(ut[:], V[:, 0, 0, :], V[:, 1, 0, :],
                                    ALU.add)
            nc.vector.tensor_tensor(ut[0:32, :], ut[0:32, :], VB[:, 0, :],
                                    ALU.add)

            # PE p-state warmup: self-contained dummy matmuls (rootw x xT into
            # a rotating scratch bank) fill PE wait gaps so real matmuls run
            # at the ramped 2.4GHz clock instead of the 1.2GHz mid p-state.
            # Conv-phase fills borrow the (then unused) pn banks; GRU-phase
            # fills borrow the (then finished) pc banks.
            _dcnt = [0]
            def pe_fill(n, tag="pc"):
                for _ in range(n):
                    Pd = ps.tile([128, HF], F32, tag=tag, bufs=2,
                                 name=f"pd{_dcnt[0]}")
                    _dcnt[0] += 1
                    mm(Pd[0:64, :], rootw, xTt[:, 0:HF], start=True, stop=True,
                       skip_group_check=True)

            # ---- conv (packed output): relu(x@rootW (+ u@Wc at s=0)) ----
            # s=1..3 emitted first: engine streams are static, the PE works on
            # the (ready) root matmuls instead of stalling at the fold gate
            xts = sb.tile([128, S, HF], BF16, tag="xts")
            for s in (1, 2, 3, 0):
                Pc = ps.tile([128, HF], F32, tag="pc", name=f"pc{s}", bufs=2)
                mm(Pc[0:64, :], rootw, xTt[:, s * NL:s * NL + HF],
                   start=True, stop=(s != 0))
                mm(Pc[64:128, :], rootw, xTt[:, s * NL + HF:(s + 1) * NL],
                   start=True, stop=(s != 0))
                if s == 0:
                    # u-term: weights zero outside rows [16t, 16t+16)
                    # (concurrent row-tiles draining the same PSUM partitions
                    # hard-fault the device)
                    for t in range(8):
                        out = Pc[64 * (t // 4):64 * (t // 4) + 64,
                                 (t % 4) * 128:(t % 4) * 128 + 128]
                        mm(out, prm[:, 768 + t * H:768 + (t + 1) * H], ut[:],
                           start=False, stop=(t % 4 == 3),
                           skip_group_check=True)
                nc.scalar.activation(xts[:, s, :], Pc[:], AF.Relu)

            # ---- GRU over s (state stored as h16 = 16*h, Whh/16 on host) ----
            ys = sb.tile([128, S, HF], BF16, tag="ys")     # h16
            # s-major fp8 state per column quarter: plane s = y8(s), plane
            # 4+s = ry8(s).  Contiguous 128-col writes (the c-major layout's
            # stride-16 byte writes cost ~5 cycles/elem).  DoubleRow k-pairs
            # are (c, c+64) within a quarter (pair stride 64 obeys the %16
            # ISA rule); separate quarter tiles so each head block depends
            # only on its own quarter's conversions.
            yqq = [sb.tile([128, 8, HF // 4], F8, tag=f"yq{t}",
                           name=f"yqq{t}") for t in range(4)]
            yqr = [t[:].rearrange("p s (b c) -> p c b s", b=2) for t in yqq]
            rt = sb.tile([128, HF], BF16, tag="rt")
            zt = sb.tile([128, HF], BF16, tag="zt")
            zc16 = sb.tile([128, HF], BF16, tag="zc16")
            ngs = sb.tile([128, HF], F32, tag="ngs")
            u_ = sb.tile([128, HF], BF16, tag="u_")
            pre = sb.tile([128, HF], BF16, tag="pre")
            mt = sb.tile([128, HF], BF16, tag="mt")

            def g_mm(P, w0, g, rhs, start, stop, skip=False):
                mm(P, prm[:, w0 + g * 128:w0 + (g + 1) * 128], rhs,
                   start=start, stop=stop, skip_group_check=skip)

            HALVES = (slice(0, HF // 2), slice(HF // 2, HF))

            def conv_fp8(s, c):
                """y8 = q(h16) on ACT, ry8 = h16 - y8 (error feedback)
                split DVE/gpsimd; all contiguous 128-col writes."""
                QW = HF // 4
                q0, q1 = c.start // QW, (c.stop + QW - 1) // QW
                for t in range(q0, q1):
                    cc = slice(t * QW, (t + 1) * QW)
                    yt = yqq[t]
                    nc.scalar.activation(yt[:, s, :], ys[:, s, cc], AF.Copy)
                    eng = nc.vector if t % 2 == 0 else nc.gpsimd
                    eng.tensor_tensor(yt[:, 4 + s, :], ys[:, s, cc],
                                      yt[:, s, :], ALU.subtract)

            def s0_step():
                Pz = ps.tile([128, HF], F32, tag="pz", name="pzs0")
                Pn = ps.tile([128, HF], F32, tag="pn", name="pns0", bufs=2)
                g_mm(Pz[:], 0, 1, xts[:, 0, :], True, True)
                g_mm(Pn[:], 0, 2, xts[:, 0, :], True, True)
                nc.scalar.activation(zt[:], Pz[:], AF.Sigmoid)
                nc.vector.tensor_scalar(zc16[:], zt[:], -16.0, 16.0,
                                        ALU.mult, ALU.add)
                # h16 = zc16*tanh(i_n); zc16 = 16*(1-z) carries the h16 scale
                nc.scalar.activation(mt[:], Pn[:], AF.Tanh)
                nc.vector.tensor_tensor(ys[:, 0, :], zc16[:], mt[:], ALU.mult)
                conv_fp8(0, slice(0, HF))

            # PE p-state warmup: self-contained dummy matmuls (rootw x xT into
            # a rotating scratch bank) fill PE wait gaps so real matmuls run
            # at the ramped 2.4GHz clock instead of the 1.2GHz mid p-state
            def gru_step(s, L, fill):
                """L = column slices (2 halves), stage-interleaved so half B
                never queues behind half A's tail on any engine.  Fresh
                per-(step,half) PSUM tiles give the tile framework the WAR
                edges (step s+1's zeroing ih matmuls must wait for step s's
                sigmoid reads)."""
                P = {}
                for i, c in enumerate(L):
                    nm = f"s{s}{'abcd'[i] if len(L) > 1 else ''}"
                    P[i] = [ps.tile([128, c.stop - c.start], F32, tag=t,
                                    name=f"{t}{nm}", bufs=b)
                            for t, b in (("pr", 2), ("pz", 1), ("pn", 2))]
                for i, c in enumerate(L):   # ih gates: only need xts
                    g_mm(P[i][0][:], 0, 0, xts[:, s, c], True, False)
                    g_mm(P[i][1][:], 0, 1, xts[:, s, c], True, False)
                for i, c in enumerate(L):   # hh gates: stall on h16[s-1]
                    g_mm(P[i][0][:], 384, 0, ys[:, s - 1, c], False, True, True)
                    g_mm(P[i][2][:], 384, 2, ys[:, s - 1, c], True, False)
                    g_mm(P[i][1][:], 384, 1, ys[:, s - 1, c], False, True, True)
                for i, c in enumerate(L):
                    nc.scalar.activation(rt[:, c], P[i][0][:], AF.Sigmoid)
                for i, c in enumerate(L):   # r * h_n in place in PSUM
                    nc.vector.tensor_tensor(P[i][2][:], rt[:, c], P[i][2][:],
                                            ALU.mult)
                for i, c in enumerate(L):   # i_n accumulates on top
                    g_mm(P[i][2][:], 0, 2, xts[:, s, c], False, True, True)
                for i, c in enumerate(L):
                    nc.scalar.activation(zt[:, c], P[i][1][:], AF.Sigmoid)
                for c in L:
                    nc.vector.tensor_scalar(zc16[:, c], zt[:, c], -16.0, 16.0,
                                            ALU.mult, ALU.add)
                for c in L:   # u = z*h16_prev off the critical chain
                    nc.gpsimd.tensor_tensor(u_[:, c], zt[:, c],
                                            ys[:, s - 1, c], ALU.mult)
                for i, c in enumerate(L):
                    nc.scalar.activation(mt[:, c], P[i][2][:], AF.Tanh)
                for c in L:   # h16 = u + zc16*tanh
                    nc.vector.tensor_tensor(pre[:, c], zc16[:, c], mt[:, c],
                                            ALU.mult)
                for c in L:
                    nc.vector.tensor_tensor(ys[:, s, c], u_[:, c], pre[:, c],
                                            ALU.add)
                for c in L:
                    conv_fp8(s, c)

            # single accumulation strip: DoubleRow and tile_position
            # col-tiling are mutually exclusive (XBUS budget)
            php = ps.tile([128, NJ], F32, tag="ph", name="php")

            def head_mms(g0, g1):
                for g in range(g0, g1):
                    mm(php[0:8, :], yqr[g // 64][:, g % 64, :, :],
                       wsb[:, g, :, 0:NJ], start=(g == 0), stop=(g == KG - 1),
                       perf_mode=PM.DoubleRow, skip_group_check=(g > 0))

            QUARTERS = [slice(i * (HF // 4), (i + 1) * (HF // 4))
                        for i in range(4)]
            s0_step()
            gru_step(1, list(HALVES), fill=0)
            gru_step(2, list(HALVES), fill=0)
            gru_step(3, QUARTERS, fill=0)
            # anchored PE warmup: these read h16[s3,A] so the scheduler keeps
            # them right before the head burst, ramping the PE clock
            for w in range(6):
                Pd = ps.tile([128, 256], F32, tag="pc", bufs=2, name=f"pw{w}")
                mm(Pd[:], prm[:, 384:512], ys[:, 3, 0:256], start=True,
                   stop=True, skip_group_check=True)
            # each head quarter depends only on its own yq tile, so the
            # accumulation chain starts as soon as quarter-1 conversions land
            head_mms(0, KG)

            psb = sb.tile([8, NJ], F32, tag="psb")
            nc.vector.tensor_copy(psb[:], php[0:8, :])
            nc.sync.dma_start(out_d, psb[:])
            if dbg_d is not None:
                nc.sync.dma_start(dbg_d, ys[:].rearrange("p s c -> p (s c)"))
                nc.sync.dma_start(dbg_x, xts[:].rearrange("p s c -> p (s c)"))
                nc.sync.dma_start(dbg_u, ut[:])

    nc.compile()
    return nc


# ----------------------------------------------------------- host data prep --
def prep_inputs(inp, dA, Dp, idxs, avals, perms):
    QA, DB = dA // 2, Dp - dA
    Q = Dp // 4
    x = np.asarray(inp["x"], dtype=np.float32)
    x0 = np.ascontiguousarray(x[0])                       # (N, 16)

    Wc = (np.asarray(inp["nn1_W2"], np.float32).reshape(FIN, H, 64)
          * np.maximum(np.asarray(inp["nn1_W1"], np.float32)[:, 0], 0.0)
          [None, None, :]).sum(-1)                        # (16, 64)

    # u-term weights: for node-block t, Wc sits at rows [16t, 16t+16) of a
    # K=128 stationary (zeros elsewhere) -> plain full-K matmuls
    wcst = np.zeros((128, 8 * H), dtype=np.float32)
    for t in range(8):
        wcst[16 * t:16 * t + FIN, t * H:(t + 1) * H] = Wc

    def bd(w):
        """gate (H,H) -> 128x(3*128) block-diagonal stationaries, cols=gates"""
        wg = np.asarray(w, np.float32).reshape(3, H, H)   # [gate, out, in]
        out = np.zeros((128, 3 * 128), dtype=np.float32)
        for g in range(3):
            out[0:64, g * 128:g * 128 + 64] = wg[g].T
            out[64:128, g * 128 + 64:(g + 1) * 128] = wg[g].T
        return out

    prm = np.zeros((128, 1344), dtype=np.float32)
    prm[:, 0:384] = bd(inp["gru_Wih"])
    prm[:, 384:768] = bd(np.asarray(inp["gru_Whh"], np.float32) / SY)
    prm[:, 768:1280] = wcst
    prm[0:FIN, 1280:1344] = np.asarray(inp["root_W"], np.float32)

    # fp8 head weights with adv residual
    Wfull = np.concatenate([np.asarray(inp["val1_W"], np.float32),
                            np.asarray(inp["adv_W"], np.float32)], axis=0)
    # (76, M, node, h); node permutation applied per core below
    Wsh = Wfull.reshape(76, M, NL, H)
    v8 = (Wsh[:64] * SW).astype(NP_F8)
    a8 = (Wsh[64:] * SW).astype(NP_F8)
    ar8 = ((Wsh[64:] - a8.astype(np.float32) / SW) * SR).astype(NP_F8)

    in_maps = []
    for c in range(M):
        pc = perms[c]
        # vg: pre-scaled gathered x0, laid out (t*16+f, level, node); tier A
        # = levels [0,dA) all nodes in 2 chunks, tier B = levels [dA,Dp)
        # partitions 0-31 (256 highest-degree nodes)
        vals = (x0[idxs[c]].reshape(Dp, 8, 128, FIN)
                * avals[c].reshape(Dp, 8, 128)[:, :, :, None])  # (Dp,t,n,f)
        arr = vals.transpose(1, 3, 0, 2).reshape(128, Dp, 128)  # (tf, q, n)
        vgA = arr[:, 0:dA, :].reshape(128, 2, QA, 128)
        vgB = arr[0:32, dA:Dp, :]                               # (32, DB, n)

        xT = x[:, c * NL:(c + 1) * NL, :][:, pc, :].transpose(2, 0, 1)

        # head: (rows, node, h) -> p = 64*half+h, k-col = nodecol;
        # [p, g, i, j]: DoubleRow pairs (c, c+64) within each 128-col quarter
        def pk(w):  # (rows, NL, H) permuted -> (128, HF, rows)
            wp = w[:, pc, :].reshape(w.shape[0], 2, HF, H)
            return wp.transpose(1, 3, 2, 0).reshape(128, HF, w.shape[0])
        wh = np.zeros((128, HF, NJP), dtype=NP_F8)
        wh[:, :, 0:64] = pk(v8[:, c])
        wh[:, :, 64:76] = pk(a8[:, c])
        wh[:, :, 76:88] = pk(ar8[:, c])
        wh = wh.reshape(128, 4, 2, 64, NJP).transpose(0, 1, 3, 2, 4)

        in_maps.append({
            "vg": _bf16(vgA.reshape(128, 2 * 128 * QA)),
            "vgb": _bf16(np.ascontiguousarray(vgB).reshape(32, 128 * DB)),
            "xT": _bf16(xT.reshape(FIN, S * NL)),
            "prm": _bf16(prm),
            "wh": np.ascontiguousarray(wh.reshape(128, KG * 2 * NJP)),
        })
    return in_maps


def head_tail(v1, advp, inp):
    """tiny fp32 head tail (<40 KFLOP) on the recombined partials"""
    v = np.maximum(v1 + np.asarray(inp["val1_b"], np.float32), 0.0)
    adv = np.maximum(advp + np.asarray(inp["adv_b"], np.float32), 0.0)
    v = np.maximum(v @ np.asarray(inp["val2_W"], np.float32).T
                   + np.asarray(inp["val2_b"], np.float32), 0.0)
    v = v @ np.asarray(inp["val3_W"], np.float32).T \
        + np.asarray(inp["val3_b"], np.float32)
    adv = adv.reshape(S, 4, 3)
    return (v[:, :, None] + adv - adv.mean(-1, keepdims=True)).astype(np.float32)


# ------------------------------------------------------------------ kernel --
def kernel(**inputs):
    global LAST_RESULTS
    inp = {k: np.asarray(v) for k, v in inputs.items()}

    # --- verify the algebraic collapse assumptions on the actual data ---
    a = inp["edge_attr"].astype(np.float32)
    W1 = inp["nn1_W1"].astype(np.float32)
    eh_ref = np.maximum(a @ W1.T + inp["nn1_b1"][None, :].astype(np.float32), 0.0)
    c1 = np.maximum(W1[:, 0], 0.0)
    ok = (np.array_equal(eh_ref, a * c1[None, :])
          and not inp["nn1_b2"].any() and not inp["conv_b"].any()
          and not inp["gru_bih"].any() and not inp["gru_bhh"].any()
          and not inp["h0"].any())
    if not ok:
        raise NotImplementedError(
            "zero-bias / rank-1 edge-MLP collapse does not hold for these inputs")

    dA, Dp, idxs, avals, perms = build_plan(inp["edge"], inp["edge_attr"])
    if (dA, Dp) not in _PROGRAM_CACHE:
        _PROGRAM_CACHE[(dA, Dp)] = build_program(dA, Dp)
    nc = _PROGRAM_CACHE[(dA, Dp)]

    in_maps = prep_inputs(inp, dA, Dp, idxs, avals, perms)
    res = run_bass_kernel_spmd(nc, in_maps, core_ids=list(range(M)))
    LAST_RESULTS = res

    parts = np.stack([r["partial"].astype(np.float32) for r in res.results])
    o88 = parts.sum(axis=0)               # (8, 88)
    ysum = o88[0:4] + o88[4:8]            # y8 + ry8 rows combined
    v1 = ysum[:, 0:64] / (SY * SW)
    advp = ysum[:, 64:76] / (SY * SW) + ysum[:, 76:88] / (SY * SR)
    return head_tail(v1, advp, inp)


# revision 80
# speedup vs baseline: 1.0741x; 1.0741x over previous
"""Trainium2 Bass kernel for nn_BHS_TEST_16724602651186 (gnn_message_passing).

Self-contained: takes FULL inputs (as from reference.setup_inputs()), shards
across 8 NeuronCores internally, returns the FULL (4,4,3) float32 output.

Math (verified against the reference semantics):
  Edge indices are < N, so NNConv message passing only affects s=0 rows.
  With nn1_b1 == 0 and edge_attr >= 0 (asserted at runtime), the edge MLP is
  exactly rank-1:  eh[e] = a_e * relu(W1),  so
      agg[n] = (sum_{e->n} a_e * x0[src_e]) @ Wc,
      Wc[f,h] = sum_k relu(W1)_k * W2[f*H+h, k]    (host-folded).
  All biases (conv_b, gru_b*, nn1_b*) are zero (asserted), h0 == 0 (asserted).
  conv_out[s] = relu(([s==0] u @ Wc) + x[s] @ root_W)
  then a 1-layer GRU over s (batch = nodes), then dueling heads.

Design (v3):
  - dst-node sharding, 1024 nodes/core; packed on-chip layout:
    partitions 0-63 = features of nodes 0-511, partitions 64-127 = features
    of nodes 512-1023 (columns = node within half).
  - Segment-sum: host pre-gathers AND pre-scales x0[src]*a into a padded
    slot grid laid out (tf-partition, chunk, node, level) so a DVE
    tensor_reduce over the innermost (level) axis yields the aggregate
    directly in (t*16+f, node) orientation = exactly the u-term moving
    operand (no transpose, no identity, no separate scale pass).
  - GRU state is stored as h16 = 16*h (Whh/16 folded on host) so the fp8
    copy of the state for the head needs no separate scaling op.
    Gates use 128-row block-diagonal stationaries (one full-width matmul
    per gate per source). n-gate: hh-matmul into PSUM, DVE multiplies by
    r in place, then the ih-matmul accumulates on top (start=False).
  - Dueling head in one fp8 DoubleRow stream: per k-pair g the moving is
    [val1_W8 (64) | adv_W8 (12) | adv_R8 (12)] e4m3 columns and the
    stationary is [y8 (4 s-cols) | ry8 (4)], ry8 = h16 - y8 (error
    feedback). 256 matmuls, out (8,88) per strip, 2 PSUM strips.
    Host recombines rows/cols with the quantization scales; adv gets full
    residual correction (adv_W e4m3 alone would cost 2.3e-2 rel err),
    val1's e4m3 error averages out through the val2/val3 layers (3.8e-3).
  - DMA: sync ring: prm, xT, vg halves (fold/GRU feed); scalar ring: the
    5.8 MB fp8 head weights in 4 chunks.
"""
import numpy as np
import ml_dtypes

import concourse.bacc as bacc
import concourse.mybir as mybir
import concourse.tile as tile
from concourse.bass_utils import run_bass_kernel_spmd

F32 = mybir.dt.float32
BF16 = mybir.dt.bfloat16
F8 = mybir.dt.float8e4
AF = mybir.ActivationFunctionType
ALU = mybir.AluOpType
PM = mybir.MatmulPerfMode
AX = mybir.AxisListType

N, FIN, H, S, E, M = 8192, 16, 64, 4, 131072, 8
NL = N // M            # 1024 dst nodes per core
HF = NL // 2           # 512 columns in packed layout
NJ = 88                # head cols: 64 val1_W8 | 12 adv_W8 | 12 adv_R8
NJP = 96               # moving pair stride padded to a 16-multiple (ISA rule)
KG = HF // 2           # 256 DoubleRow k-pair groups per core
SW, SR, SY = 256.0, 4096.0, 16.0   # W8, adv residual, ys scales

NP_F8 = ml_dtypes.float8_e4m3
LAST_RESULTS = None    # BassKernelResults of the most recent run (for test.py)
_PROGRAM_CACHE = {}
_DEBUG_YS = False


def _bf16(x):
    return np.ascontiguousarray(np.asarray(x, dtype=np.float32)).astype(
        ml_dtypes.bfloat16)


def _f8(x):
    return np.ascontiguousarray(np.asarray(x, dtype=np.float32)).astype(NP_F8)


# ---------------------------------------------------------------- host plan --
def build_plan(edge, edge_attr):
    """Slot layout: for each core, slot level j, node n: the j-th in-edge of
    node n (src index + edge_attr), zero-filled.  Nodes are permuted by
    descending in-degree so the slot grid can be 2-tiered: levels [0, dA)
    cover all nodes, levels [dA, Dp) only the 256 highest-degree nodes
    (blocks 0-1 = partitions 0-31 in tf packing).  The permutation is
    applied consistently to xT and the head weights; the output is a sum
    over nodes so nothing needs un-permuting."""
    src = np.asarray(edge[0], dtype=np.int64)
    dst = np.asarray(edge[1], dtype=np.int64)
    a = np.asarray(edge_attr[:, 0], dtype=np.float32)

    percore = []
    perms = np.zeros((M, NL), dtype=np.int64)
    dA_req, D_req = 1, 1
    for c in range(M):
        lo = c * NL
        mask = (dst >= lo) & (dst < lo + NL)
        src_c, a_c, dstl = src[mask], a[mask], dst[mask] - lo
        deg = np.bincount(dstl, minlength=NL)
        perm = np.argsort(-deg, kind="stable")
        inv = np.empty(NL, dtype=np.int64)
        inv[perm] = np.arange(NL)
        perms[c] = perm
        degs = deg[perm]
        dA_req = max(dA_req, int(degs[256:].max()))
        D_req = max(D_req, int(degs.max()))
        percore.append((src_c, a_c, inv[dstl]))

    dA = (dA_req + 1) // 2 * 2
    Dp = max((D_req + 1) // 2 * 2, dA + 2)

    idxs = np.zeros((M, Dp, NL), dtype=np.int16)
    avals = np.zeros((M, Dp, NL), dtype=np.float32)
    for c in range(M):
        src_c, a_c, dstl = percore[c]
        order = np.argsort(dstl, kind="stable")
        ds = dstl[order]
        starts = np.searchsorted(ds, np.arange(NL))
        occ = np.arange(len(ds)) - starts[ds]
        idxs[c, occ, ds] = src_c[order].astype(np.int16)
        avals[c, occ, ds] = a_c[order]
    return dA, Dp, idxs, avals, perms


# ------------------------------------------------------------- bass program --
def build_program(dA, Dp):
    QA = dA // 2           # levels per tier-A vg chunk (2 chunks)
    DB = Dp - dA           # tier-B levels (blocks 0-1 / partitions 0-31 only)
    nc = bacc.Bacc("TRN2", target_bir_lowering=False, debug=False,
                   num_devices=M, num_swdge_queues=1)
    d = {}
    def din(name, shape, dt=BF16):
        d[name] = nc.dram_tensor(name, list(shape), dt, kind="ExternalInput").ap()
    din("vg", (128, 2 * 128 * QA))
    din("vgb", (32, 128 * DB))
    din("xT", (FIN, S * NL))
    din("prm", (128, 1344))   # [ih-bd(384) | hh-bd/16(384) | wcst(512) | rootw rows0:16 (64)]
    din("wh", (128, KG * 2 * NJP), F8)
    out_d = nc.dram_tensor("partial", [8, NJ], F32, kind="ExternalOutput").ap()
    dbg_d = nc.dram_tensor("dbg_ys", [128, S * HF], mybir.dt.bfloat16,
                           kind="ExternalOutput").ap() if _DEBUG_YS else None
    dbg_x = nc.dram_tensor("dbg_xts", [128, S * HF], mybir.dt.bfloat16,
                           kind="ExternalOutput").ap() if _DEBUG_YS else None
    dbg_u = nc.dram_tensor("dbg_ut", [128, 128], mybir.dt.bfloat16,
                           kind="ExternalOutput").ap() if _DEBUG_YS else None

    mm = nc.tensor.matmul

    with tile.TileContext(nc) as tc:
        with (
            tc.tile_pool(name="sb", bufs=1) as sb,
            tc.tile_pool(name="ps", bufs=1, space="PSUM") as ps,
        ):
            # ---- scalar ring (the fast one, ~390GB/s vs sync's ~130): vg
            # tiers first (ys critical path), then the fp8 head weights
            VB = sb.tile([32, DB, 128], BF16, tag="VB")
            nc.scalar.dma_start(VB[:].rearrange("p q n -> p (q n)"), d["vgb"])
            V = sb.tile([128, 2, QA, 128], BF16, tag="V")
            Vf = V[:].rearrange("p c q n -> p (c q n)")
            CW = 128 * QA
            for c in range(2):
                nc.scalar.dma_start(Vf[:, c * CW:(c + 1) * CW],
                                    d["vg"][:, c * CW:(c + 1) * CW])
            # single DMA: more chunks stall the scalar engine on HWDGE ring
            # credits, delaying the conv relus behind the issue instructions
            wsb = sb.tile([128, KG, 2, NJP], F8, tag="wsb")
            wf = wsb[:].rearrange("p g i j -> p (g i j)")
            nc.scalar.dma_start(wf[:], d["wh"][:])

            # ---- sync ring: prm + xT (small, needed by conv/GRU)
            prm = sb.tile([128, 1344], BF16, tag="prm")
            nc.sync.dma_start(prm[:], d["prm"])
            xTt = sb.tile([FIN, S * NL], BF16, tag="xT")
            nc.sync.dma_start(xTt[:], d["xT"])
            rootw = prm[0:16, 1280:1344]

            # dummy 1-col Tanh pulls the ACT_TABLE_LOAD for the tanh table
            # into the idle DMA-wait window instead of mid-GRU
            warm = sb.tile([128, 4], BF16, tag="warm")
            nc.vector.memset(warm[:], 0.0)
            nc.scalar.activation(warm[:, 0:1], warm[:, 1:2], AF.Tanh)
            nc.scalar.activation(warm[:, 2:3], warm[:, 3:4], AF.Sigmoid)

            # ---- fold: bf16 trees over the level (outer) dim so every add
            # runs on contiguous node columns in DVE 2x mode.  Tier B (the
            # 256 highest-degree nodes, partitions 0-31) folds first, then
            # tier A's two chunks, then the cross adds.
            cur = DB
            while cur > 1:
                half = (cur + 1) // 2
                nch = cur - half
                nc.vector.tensor_tensor(VB[:, 0:nch, :], VB[:, 0:nch, :],
                                        VB[:, half:half + nch, :], ALU.add)
                cur = half
            for c in range(2):
                cur = QA
                while cur > 1:
                    half = (cur + 1) // 2
                    nch = cur - half
                    nc.vector.tensor_tensor(V[:, c, 0:nch, :], V[:, c, 0:nch, :],
                                            V[:, c, half:half + nch, :], ALU.add)
                    cur = half
            ut = sb.tile([128, 128], BF16, tag="ut")
            nc.vector.tensor_tensor(ut[:], V[:, 0, 0, :], V[:, 1, 0, :],
                                    ALU.add)
            nc.vector.tensor_tensor(ut[0:32, :], ut[0:32, :], VB[:, 0, :],
                                    ALU.add)

            # PE p-state warmup: self-contained dummy matmuls (rootw x xT into
            # a rotating scratch bank) fill PE wait gaps so real matmuls run
            # at the ramped 2.4GHz clock instead of the 1.2GHz mid p-state.
            # Conv-phase fills borrow the (then unused) pn banks; GRU-phase
            # fills borrow the (then finished) pc banks.
            _dcnt = [0]
            def pe_fill(n, tag="pc"):
                for _ in range(n):
                    Pd = ps.tile([128, HF], F32, tag=tag, bufs=2,
                                 name=f"pd{_dcnt[0]}")
                    _dcnt[0] += 1
                    mm(Pd[0:64, :], rootw, xTt[:, 0:HF], start=True, stop=True,
                       skip_group_check=True)

            # ---- conv (packed output): relu(x@rootW (+ u@Wc at s=0)) ----
            # s=1..3 emitted first: engine streams are static, the PE works on
            # the (ready) root matmuls instead of stalling at the fold gate
            xts = sb.tile([128, S, HF], BF16, tag="xts")
            for s in (1, 2, 3, 0):
                Pc = ps.tile([128, HF], F32, tag="pc", name=f"pc{s}", bufs=2)
                mm(Pc[0:64, :], rootw, xTt[:, s * NL:s * NL + HF],
                   start=True, stop=(s != 0))
                mm(Pc[64:128, :], rootw, xTt[:, s * NL + HF:(s + 1) * NL],
                   start=True, stop=(s != 0))
                if s == 0:
                    # u-term: weights zero outside rows [16t, 16t+16)
                    # (concurrent row-tiles draining the same PSUM partitions
                    # hard-fault the device)
                    for t in range(8):
                        out = Pc[64 * (t // 4):64 * (t // 4) + 64,
                                 (t % 4) * 128:(t % 4) * 128 + 128]
                        mm(out, prm[:, 768 + t * H:768 + (t + 1) * H], ut[:],
                           start=False, stop=(t % 4 == 3),
                           skip_group_check=True)
                nc.scalar.activation(xts[:, s, :], Pc[:], AF.Relu)

            # ---- GRU over s (state stored as h16 = 16*h, Whh/16 on host) ----
            ys = sb.tile([128, S, HF], BF16, tag="ys")     # h16
            # s-major fp8 state per column quarter: plane s = y8(s), plane
            # 4+s = ry8(s).  Contiguous 128-col writes (the c-major layout's
            # stride-16 byte writes cost ~5 cycles/elem).  DoubleRow k-pairs
            # are (c, c+64) within a quarter (pair stride 64 obeys the %16
            # ISA rule); separate quarter tiles so each head block depends
            # only on its own quarter's conversions.
            yqq = [sb.tile([128, 8, HF // 4], F8, tag=f"yq{t}",
                           name=f"yqq{t}") for t in range(4)]
            yqr = [t[:].rearrange("p s (b c) -> p c b s", b=2) for t in yqq]
            rt = sb.tile([128, HF], BF16, tag="rt")
            zt = sb.tile([128, HF], BF16, tag="zt")
            zc16 = sb.tile([128, HF], BF16, tag="zc16")
            ngs = sb.tile([128, HF], F32, tag="ngs")
            u_ = sb.tile([128, HF], BF16, tag="u_")
            pre = sb.tile([128, HF], BF16, tag="pre")
            mt = sb.tile([128, HF], BF16, tag="mt")

            def g_mm(P, w0, g, rhs, start, stop, skip=False):
                mm(P, prm[:, w0 + g * 128:w0 + (g + 1) * 128], rhs,
                   start=start, stop=stop, skip_group_check=skip)

            HALVES = (slice(0, HF // 2), slice(HF // 2, HF))

            def conv_fp8(s, c):
                """y8 = q(h16) on ACT, ry8 = h16 - y8 (error feedback)
                split DVE/gpsimd; all contiguous 128-col writes."""
                QW = HF // 4
                q0, q1 = c.start // QW, (c.stop + QW - 1) // QW
                for t in range(q0, q1):
                    cc = slice(t * QW, (t + 1) * QW)
                    yt = yqq[t]
                    if s < S - 1 and t % 2 == 0:
                        # steady-state steps: offload to the idle gpsimd
                        # (ACT is the binding engine through the GRU)
                        nc.gpsimd.tensor_copy(yt[:, s, :], ys[:, s, cc])
                        nc.gpsimd.tensor_tensor(yt[:, 4 + s, :], ys[:, s, cc],
                                                yt[:, s, :], ALU.subtract)
                    else:
                        nc.scalar.activation(yt[:, s, :], ys[:, s, cc],
                                             AF.Copy)
                        eng = nc.vector if t % 2 == 0 else nc.gpsimd
                        eng.tensor_tensor(yt[:, 4 + s, :], ys[:, s, cc],
                                          yt[:, s, :], ALU.subtract)

            def s0_step():
                Pz = ps.tile([128, HF], F32, tag="pz", name="pzs0")
                Pn = ps.tile([128, HF], F32, tag="pn", name="pns0", bufs=2)
                g_mm(Pz[:], 0, 1, xts[:, 0, :], True, True)
                g_mm(Pn[:], 0, 2, xts[:, 0, :], True, True)
                nc.scalar.activation(zt[:], Pz[:], AF.Sigmoid)
                nc.vector.tensor_scalar(zc16[:], zt[:], -16.0, 16.0,
                                        ALU.mult, ALU.add)
                # h16 = zc16*tanh(i_n); zc16 = 16*(1-z) carries the h16 scale
                nc.scalar.activation(mt[:], Pn[:], AF.Tanh)
                nc.vector.tensor_tensor(ys[:, 0, :], zc16[:], mt[:], ALU.mult)
                conv_fp8(0, slice(0, HF))

            # PE p-state warmup: self-contained dummy matmuls (rootw x xT into
            # a rotating scratch bank) fill PE wait gaps so real matmuls run
            # at the ramped 2.4GHz clock instead of the 1.2GHz mid p-state
            def gru_step(s, L, fill):
                """L = column slices (2 halves), stage-interleaved so half B
                never queues behind half A's tail on any engine.  Fresh
                per-(step,half) PSUM tiles give the tile framework the WAR
                edges (step s+1's zeroing ih matmuls must wait for step s's
                sigmoid reads)."""
                P = {}
                for i, c in enumerate(L):
                    nm = f"s{s}{'abcd'[i] if len(L) > 1 else ''}"
                    P[i] = [ps.tile([128, c.stop - c.start], F32, tag=t,
                                    name=f"{t}{nm}", bufs=b)
                            for t, b in (("pr", 2), ("pz", 1), ("pn", 2))]
                for i, c in enumerate(L):   # ih gates: only need xts
                    g_mm(P[i][0][:], 0, 0, xts[:, s, c], True, False)
                    g_mm(P[i][1][:], 0, 1, xts[:, s, c], True, False)
                for i, c in enumerate(L):   # hh gates: stall on h16[s-1]
                    g_mm(P[i][0][:], 384, 0, ys[:, s - 1, c], False, True, True)
                    g_mm(P[i][2][:], 384, 2, ys[:, s - 1, c], True, False)
                    g_mm(P[i][1][:], 384, 1, ys[:, s - 1, c], False, True, True)
                for i, c in enumerate(L):
                    nc.scalar.activation(rt[:, c], P[i][0][:], AF.Sigmoid)
                for i, c in enumerate(L):   # r * h_n in place in PSUM
                    nc.vector.tensor_tensor(P[i][2][:], rt[:, c], P[i][2][:],
                                            ALU.mult)
                for i, c in enumerate(L):   # i_n accumulates on top
                    g_mm(P[i][2][:], 0, 2, xts[:, s, c], False, True, True)
                for i, c in enumerate(L):
                    nc.scalar.activation(zt[:, c], P[i][1][:], AF.Sigmoid)
                for c in L:
                    nc.vector.tensor_scalar(zc16[:, c], zt[:, c], -16.0, 16.0,
                                            ALU.mult, ALU.add)
                for c in L:   # u = z*h16_prev off the critical chain
                    nc.gpsimd.tensor_tensor(u_[:, c], zt[:, c],
                                            ys[:, s - 1, c], ALU.mult)
                for i, c in enumerate(L):
                    nc.scalar.activation(mt[:, c], P[i][2][:], AF.Tanh)
                for c in L:   # h16 = u + zc16*tanh
                    nc.vector.tensor_tensor(pre[:, c], zc16[:, c], mt[:, c],
                                            ALU.mult)
                for c in L:
                    nc.vector.tensor_tensor(ys[:, s, c], u_[:, c], pre[:, c],
                                            ALU.add)
                for c in L:
                    conv_fp8(s, c)

            # single accumulation strip: DoubleRow and tile_position
            # col-tiling are mutually exclusive (XBUS budget)
            php = ps.tile([128, NJ], F32, tag="ph", name="php")

            def head_mms(g0, g1):
                for g in range(g0, g1):
                    mm(php[0:8, :], yqr[g // 64][:, g % 64, :, :],
                       wsb[:, g, :, 0:NJ], start=(g == 0), stop=(g == KG - 1),
                       perf_mode=PM.DoubleRow, skip_group_check=(g > 0))

            QUARTERS = [slice(i * (HF // 4), (i + 1) * (HF // 4))
                        for i in range(4)]
            s0_step()
            gru_step(1, list(HALVES), fill=0)
            gru_step(2, list(HALVES), fill=0)
            gru_step(3, QUARTERS, fill=0)
            # anchored PE warmup: these read h16[s3,A] so the scheduler keeps
            # them right before the head burst, ramping the PE clock
            for w in range(6):
                Pd = ps.tile([128, 256], F32, tag="pc", bufs=2, name=f"pw{w}")
                mm(Pd[:], prm[:, 384:512], ys[:, 3, 0:256], start=True,
                   stop=True, skip_group_check=True)
            # each head quarter depends only on its own yq tile, so the
            # accumulation chain starts as soon as quarter-1 conversions land
            head_mms(0, KG)

            psb = sb.tile([8, NJ], F32, tag="psb")
            nc.vector.tensor_copy(psb[:], php[0:8, :])
            nc.sync.dma_start(out_d, psb[:])
            if dbg_d is not None:
                nc.sync.dma_start(dbg_d, ys[:].rearrange("p s c -> p (s c)"))
                nc.sync.dma_start(dbg_x, xts[:].rearrange("p s c -> p (s c)"))
                nc.sync.dma_start(dbg_u, ut[:])

    nc.compile()
    return nc


# ----------------------------------------------------------- host data prep --
def prep_inputs(inp, dA, Dp, idxs, avals, perms):
    QA, DB = dA // 2, Dp - dA
    Q = Dp // 4
    x = np.asarray(inp["x"], dtype=np.float32)
    x0 = np.ascontiguousarray(x[0])                       # (N, 16)

    Wc = (np.asarray(inp["nn1_W2"], np.float32).reshape(FIN, H, 64)
          * np.maximum(np.asarray(inp["nn1_W1"], np.float32)[:, 0], 0.0)
          [None, None, :]).sum(-1)                        # (16, 64)

    # u-term weights: for node-block t, Wc sits at rows [16t, 16t+16) of a
    # K=128 stationary (zeros elsewhere) -> plain full-K matmuls
    wcst = np.zeros((128, 8 * H), dtype=np.float32)
    for t in range(8):
        wcst[16 * t:16 * t + FIN, t * H:(t + 1) * H] = Wc

    def bd(w):
        """gate (H,H) -> 128x(3*128) block-diagonal stationaries, cols=gates"""
        wg = np.asarray(w, np.float32).reshape(3, H, H)   # [gate, out, in]
        out = np.zeros((128, 3 * 128), dtype=np.float32)
        for g in range(3):
            out[0:64, g * 128:g * 128 + 64] = wg[g].T
            out[64:128, g * 128 + 64:(g + 1) * 128] = wg[g].T
        return out

    prm = np.zeros((128, 1344), dtype=np.float32)
    prm[:, 0:384] = bd(inp["gru_Wih"])
    prm[:, 384:768] = bd(np.asarray(inp["gru_Whh"], np.float32) / SY)
    prm[:, 768:1280] = wcst
    prm[0:FIN, 1280:1344] = np.asarray(inp["root_W"], np.float32)

    # fp8 head weights with adv residual
    Wfull = np.concatenate([np.asarray(inp["val1_W"], np.float32),
                            np.asarray(inp["adv_W"], np.float32)], axis=0)
    # (76, M, node, h); node permutation applied per core below
    Wsh = Wfull.reshape(76, M, NL, H)
    v8 = (Wsh[:64] * SW).astype(NP_F8)
    a8 = (Wsh[64:] * SW).astype(NP_F8)
    ar8 = ((Wsh[64:] - a8.astype(np.float32) / SW) * SR).astype(NP_F8)

    in_maps = []
    for c in range(M):
        pc = perms[c]
        # vg: pre-scaled gathered x0, laid out (t*16+f, level, node); tier A
        # = levels [0,dA) all nodes in 2 chunks, tier B = levels [dA,Dp)
        # partitions 0-31 (256 highest-degree nodes)
        vals = (x0[idxs[c]].reshape(Dp, 8, 128, FIN)
                * avals[c].reshape(Dp, 8, 128)[:, :, :, None])  # (Dp,t,n,f)
        arr = vals.transpose(1, 3, 0, 2).reshape(128, Dp, 128)  # (tf, q, n)
        vgA = arr[:, 0:dA, :].reshape(128, 2, QA, 128)
        vgB = arr[0:32, dA:Dp, :]                               # (32, DB, n)

        xT = x[:, c * NL:(c + 1) * NL, :][:, pc, :].transpose(2, 0, 1)

        # head: (rows, node, h) -> p = 64*half+h, k-col = nodecol;
        # [p, g, i, j]: DoubleRow pairs (c, c+64) within each 128-col quarter
        def pk(w):  # (rows, NL, H) permuted -> (128, HF, rows)
            wp = w[:, pc, :].reshape(w.shape[0], 2, HF, H)
            return wp.transpose(1, 3, 2, 0).reshape(128, HF, w.shape[0])
        wh = np.zeros((128, HF, NJP), dtype=NP_F8)
        wh[:, :, 0:64] = pk(v8[:, c])
        wh[:, :, 64:76] = pk(a8[:, c])
        wh[:, :, 76:88] = pk(ar8[:, c])
        wh = wh.reshape(128, 4, 2, 64, NJP).transpose(0, 1, 3, 2, 4)

        in_maps.append({
            "vg": _bf16(vgA.reshape(128, 2 * 128 * QA)),
            "vgb": _bf16(np.ascontiguousarray(vgB).reshape(32, 128 * DB)),
            "xT": _bf16(xT.reshape(FIN, S * NL)),
            "prm": _bf16(prm),
            "wh": np.ascontiguousarray(wh.reshape(128, KG * 2 * NJP)),
        })
    return in_maps


def head_tail(v1, advp, inp):
    """tiny fp32 head tail (<40 KFLOP) on the recombined partials"""
    v = np.maximum(v1 + np.asarray(inp["val1_b"], np.float32), 0.0)
    adv = np.maximum(advp + np.asarray(inp["adv_b"], np.float32), 0.0)
    v = np.maximum(v @ np.asarray(inp["val2_W"], np.float32).T
                   + np.asarray(inp["val2_b"], np.float32), 0.0)
    v = v @ np.asarray(inp["val3_W"], np.float32).T \
        + np.asarray(inp["val3_b"], np.float32)
    adv = adv.reshape(S, 4, 3)
    return (v[:, :, None] + adv - adv.mean(-1, keepdims=True)).astype(np.float32)


# ------------------------------------------------------------------ kernel --
def kernel(**inputs):
    global LAST_RESULTS
    inp = {k: np.asarray(v) for k, v in inputs.items()}

    # --- verify the algebraic collapse assumptions on the actual data ---
    a = inp["edge_attr"].astype(np.float32)
    W1 = inp["nn1_W1"].astype(np.float32)
    eh_ref = np.maximum(a @ W1.T + inp["nn1_b1"][None, :].astype(np.float32), 0.0)
    c1 = np.maximum(W1[:, 0], 0.0)
    ok = (np.array_equal(eh_ref, a * c1[None, :])
          and not inp["nn1_b2"].any() and not inp["conv_b"].any()
          and not inp["gru_bih"].any() and not inp["gru_bhh"].any()
          and not inp["h0"].any())
    if not ok:
        raise NotImplementedError(
            "zero-bias / rank-1 edge-MLP collapse does not hold for these inputs")

    dA, Dp, idxs, avals, perms = build_plan(inp["edge"], inp["edge_attr"])
    if (dA, Dp) not in _PROGRAM_CACHE:
        _PROGRAM_CACHE[(dA, Dp)] = build_program(dA, Dp)
    nc = _PROGRAM_CACHE[(dA, Dp)]

    in_maps = prep_inputs(inp, dA, Dp, idxs, avals, perms)
    res = run_bass_kernel_spmd(nc, in_maps, core_ids=list(range(M)))
    LAST_RESULTS = res

    parts = np.stack([r["partial"].astype(np.float32) for r in res.results])
    o88 = parts.sum(axis=0)               # (8, 88)
    ysum = o88[0:4] + o88[4:8]            # y8 + ry8 rows combined
    v1 = ysum[:, 0:64] / (SY * SW)
    advp = ysum[:, 64:76] / (SY * SW) + ysum[:, 76:88] / (SY * SR)
    return head_tail(v1, advp, inp)
